# revision 6
# baseline (speedup 1.0000x reference)
# Trainium2 Bass kernel for dense soft-MoE (nn_MANN_78726750536045).
#
# Math (per sample b):
#   gates = softmax(MLP_elu(x_gate))                     [K=8]
#   h0 = elu(sum_k g_k * (x_main @ W1_k.T) + gates@eb1)  [512]
#   h1 = elu(sum_k g_k * (h0 @ W2_k.T) + gates@eb2)      [512]
#   out =     sum_k g_k * (h1 @ W3_k.T) + gates@eb3      [512]
#
# Key transformation: g_k * (h @ Wk.T) == ((g_k * h) @ Wk.T), so each expert
# layer becomes ONE dense GEMM with contraction dim K*512 = 4096 over the
# "gated-replicated" activation X'[(k,i), b] = g[b,k] * h[b,i].  X' is built
# on the tensor engine itself via per-expert diagonal matrices:
#   X'_chunk = h_chunk.T @ diag(g_k)   (fuses the transpose AND the gating).
# The bias gates@eb is folded in as one extra 8-row chunk of the same GEMM.
#
# Sharding: pure data-parallel, batch 1024 -> 128 rows per core x 8 cores.
# Expert weights are cast to bf16 on host (HBM traffic is the bottleneck:
# ~12.3MB/core bf16 @ ~360GB/s ~= 34us); gating network stays fp32.

import numpy as np
import ml_dtypes

B = 1024
X_MAIN, X_GATE, HID, Y_DIM, GHID, K = 480, 128, 512, 512, 32, 8
NCORES = 8
BL = B // NCORES  # 128 rows per core
P = 128
NCH = 32  # contraction chunks per expert layer (K * 512 / 128)

_cache = {}


def _build_nc():
    from contextlib import ExitStack

    import concourse.bacc as bacc
    import concourse.mybir as mybir
    import concourse.tile as tile
    from concourse.bass import ts

    f32 = mybir.dt.float32
    bf16 = mybir.dt.bfloat16
    AF = mybir.ActivationFunctionType
    OP = mybir.AluOpType

    nc = bacc.Bacc("TRN2", target_bir_lowering=False, debug=False)

    # ---- DRAM I/O ----
    d_xgT = nc.dram_tensor("xgT", [X_GATE, BL], f32, kind="ExternalInput")
    d_xm = nc.dram_tensor("xm", [BL, HID], bf16, kind="ExternalInput")
    d_g1T = nc.dram_tensor("g1T", [X_GATE, GHID], f32, kind="ExternalInput")
    d_g2T = nc.dram_tensor("g2T", [GHID, GHID], f32, kind="ExternalInput")
    d_g3T = nc.dram_tensor("g3T", [GHID, K], f32, kind="ExternalInput")
    d_gb1 = nc.dram_tensor("gb1", [GHID, 1], f32, kind="ExternalInput")
    d_gb2 = nc.dram_tensor("gb2", [GHID, 1], f32, kind="ExternalInput")
    d_gb3 = nc.dram_tensor("gb3", [K, 1], f32, kind="ExternalInput")
    d_idf = nc.dram_tensor("idf", [P, P], f32, kind="ExternalInput")
    d_w = [
        nc.dram_tensor(f"w{l}", [NCH * P, HID], bf16, kind="ExternalInput")
        for l in range(3)
    ]
    d_be = [
        nc.dram_tensor(f"be{l}", [K, HID], bf16, kind="ExternalInput")
        for l in range(3)
    ]
    d_out = nc.dram_tensor("out", [BL, Y_DIM], f32, kind="ExternalOutput")

    with ExitStack() as ctx:
        tc = ctx.enter_context(tile.TileContext(nc))
        consts = ctx.enter_context(tc.tile_pool(name="consts", bufs=1))
        sb = ctx.enter_context(tc.tile_pool(name="sb", bufs=2))
        wpool = ctx.enter_context(tc.tile_pool(name="wp", bufs=16))
        xpp = ctx.enter_context(tc.tile_pool(name="xpp", bufs=2))
        pmain = ctx.enter_context(tc.tile_pool(name="pmain", bufs=2, space="PSUM"))
        ppx = ctx.enter_context(tc.tile_pool(name="ppx", bufs=4, space="PSUM"))
        pg = ctx.enter_context(tc.tile_pool(name="pg", bufs=2, space="PSUM"))

        # ---- load constants ----
        t_xgT = consts.tile([X_GATE, BL], f32)
        nc.sync.dma_start(t_xgT, d_xgT[:])
        t_xm = consts.tile([BL, HID], bf16)
        nc.sync.dma_start(t_xm, d_xm[:])
        t_g1T = consts.tile([X_GATE, GHID], f32)
        nc.sync.dma_start(t_g1T, d_g1T[:])
        t_g2T = consts.tile([GHID, GHID], f32)
        nc.sync.dma_start(t_g2T, d_g2T[:])
        t_g3T = consts.tile([GHID, K], f32)
        nc.sync.dma_start(t_g3T, d_g3T[:])
        t_gb1 = consts.tile([GHID, 1], f32)
        nc.sync.dma_start(t_gb1, d_gb1[:])
        t_gb2 = consts.tile([GHID, 1], f32)
        nc.sync.dma_start(t_gb2, d_gb2[:])
        t_gb3 = consts.tile([K, 1], f32)
        nc.sync.dma_start(t_gb3, d_gb3[:])
        t_idf = consts.tile([P, P], f32)
        nc.sync.dma_start(t_idf, d_idf[:])
        t_be = []
        for l in range(3):
            tb = consts.tile([K, HID], bf16, name=f"be{l}")
            nc.sync.dma_start(tb, d_be[l][:])
            t_be.append(tb)

        # ---- gating network (fp32, [feature, batch] layout) ----
        def elu_block(p_in, bias, width):
            e = sb.tile([width, BL], f32, tag="gelu_e")
            nc.scalar.activation(e, p_in, AF.Exp, bias=bias)
            r = sb.tile([width, BL], f32, tag="gelu_r")
            nc.vector.tensor_scalar(r, p_in, bias, 0.0, OP.add, OP.max)
            t = sb.tile([width, BL], f32, tag="gelu_t")
            nc.vector.tensor_scalar(t, e, -1.0, 0.0, OP.add, OP.min)
            g = sb.tile([width, BL], f32, tag="gelu_g")
            nc.vector.tensor_tensor(g, r, t, OP.add)
            return g

        p1 = pg.tile([GHID, BL], f32, tag="pg")
        nc.tensor.matmul(p1, t_g1T, t_xgT, start=True, stop=True)
        g1 = elu_block(p1, t_gb1, GHID)

        p2 = pg.tile([GHID, BL], f32, tag="pg")
        nc.tensor.matmul(p2, t_g2T, g1, start=True, stop=True)
        g2 = elu_block(p2, t_gb2, GHID)

        p3 = pg.tile([K, BL], f32, tag="pg")
        nc.tensor.matmul(p3, t_g3T, g2, start=True, stop=True)

        # softmax over K (partition dim): exp -> transpose [K,BL]->[BL,K]
        # -> free-dim sum + reciprocal + scale.
        es = sb.tile([K, BL], f32)
        nc.scalar.activation(es, p3, AF.Exp, bias=t_gb3)
        p_esT = pg.tile([BL, K], f32, tag="pg")
        nc.tensor.transpose(p_esT, es, t_idf[0:K, 0:K])
        ssum = sb.tile([BL, 1], f32)
        nc.vector.tensor_reduce(ssum, p_esT, mybir.AxisListType.X, OP.add)
        recip = sb.tile([BL, 1], f32)
        nc.vector.reciprocal(recip, ssum)
        gates = sb.tile([BL, K], f32)  # [b, k] fp32
        nc.vector.tensor_scalar(gates, p_esT, recip, None, OP.mult)

        # gates^T [K, BL] in bf16 (for the bias chunk of the main GEMM)
        p_gT = pg.tile([K, BL], f32, tag="pg")
        nc.tensor.transpose(p_gT, gates, t_idf)
        gT = sb.tile([K, BL], bf16)
        nc.vector.tensor_copy(gT, p_gT)

        # per-expert diagonal matrices diag(g[:,k]), bf16, built once
        diags = consts.tile([P, K * P], bf16)
        for k in range(K):
            nc.vector.tensor_scalar(
                diags[:, ts(k, P)], t_idf, gates[:, k : k + 1], None, OP.mult
            )

        # ---- three expert layers ----
        h = t_xm
        for l in range(3):
            # Build X' [(j,k,il), b] in SBUF: for each input-block j (128 wide)
            # and each group g of 4 experts, one matmul
            #   px = h_j.T @ [diag_{4g} .. diag_{4g+3}]  -> [128, 512]
            XP = xpp.tile([P, NCH * P], bf16, tag="XP")
            for j in range(4):
                for g in range(2):
                    px = ppx.tile([P, 512], f32, tag="px")
                    nc.tensor.matmul(
                        px,
                        h[:, ts(j, P)],
                        diags[:, g * 512 : (g + 1) * 512],
                        start=True,
                        stop=True,
                    )
                    off = (j * 8 + g * 4) * P
                    nc.vector.tensor_copy(XP[:, off : off + 512], px)

            # Main GEMM: 32 accumulating chunk matmuls + 1 bias chunk
            pm = pmain.tile([P, HID], f32, tag="pm")
            for c in range(NCH):
                wt = wpool.tile([P, HID], bf16, tag="wt")
                nc.sync.dma_start(wt, d_w[l][ts(c, P), :])
                nc.tensor.matmul(
                    pm, XP[:, ts(c, P)], wt, start=(c == 0), stop=False
                )
            nc.tensor.matmul(pm, gT, t_be[l], start=False, stop=True)

            if l < 2:
                # ELU: relu(x) + min(exp(x)-1, 0), output bf16
                e = sb.tile([P, HID], f32, tag="ee")
                nc.scalar.activation(e, pm, AF.Exp)
                r = sb.tile([P, HID], f32, tag="er")
                nc.scalar.activation(r, pm, AF.Relu)
                t = sb.tile([P, HID], f32, tag="et")
                nc.vector.tensor_scalar(t, e, -1.0, 0.0, OP.add, OP.min)
                h2 = sb.tile([P, HID], bf16, tag="eh")
                nc.vector.tensor_tensor(h2, r, t, OP.add)
                h = h2
            else:
                o = sb.tile([P, Y_DIM], f32)
                nc.vector.tensor_copy(o, pm)
                nc.sync.dma_start(d_out[:], o)

    nc.compile()
    return nc


def _prep_inputs(inputs):
    bf16 = ml_dtypes.bfloat16
    xm = np.asarray(inputs["x_main"], np.float32)
    xg = np.asarray(inputs["x_gate"], np.float32)

    xgT = np.ascontiguousarray(xg.T)  # [128, B]
    xmp = np.zeros((B, HID), np.float32)
    xmp[:, :X_MAIN] = xm
    xmp = xmp.astype(bf16)

    g1T = np.ascontiguousarray(np.asarray(inputs["gw1"], np.float32).T)
    g2T = np.ascontiguousarray(np.asarray(inputs["gw2"], np.float32).T)
    g3T = np.ascontiguousarray(np.asarray(inputs["gw3"], np.float32).T)
    gb1 = np.asarray(inputs["gb1"], np.float32).reshape(GHID, 1)
    gb2 = np.asarray(inputs["gb2"], np.float32).reshape(GHID, 1)
    gb3 = np.asarray(inputs["gb3"], np.float32).reshape(K, 1)
    idf = np.eye(P, dtype=np.float32)

    # expert weights -> [(j, k, il), o] chunk layout, bf16, input dim padded
    def pack_w(ew):
        ewt = np.asarray(ew, np.float32).transpose(0, 2, 1)  # [K, in, out]
        if ewt.shape[1] < HID:
            pad = np.zeros((K, HID, ewt.shape[2]), np.float32)
            pad[:, : ewt.shape[1], :] = ewt
            ewt = pad
        w = ewt.reshape(K, 4, P, HID).transpose(1, 0, 2, 3).reshape(NCH * P, HID)
        return np.ascontiguousarray(w.astype(bf16))

    w = [pack_w(inputs["ew1"]), pack_w(inputs["ew2"]), pack_w(inputs["ew3"])]
    be = [
        np.asarray(inputs[f"eb{l + 1}"], np.float32).astype(bf16) for l in range(3)
    ]

    shared = {
        "g1T": g1T, "g2T": g2T, "g3T": g3T,
        "gb1": gb1, "gb2": gb2, "gb3": gb3, "idf": idf,
        "w0": w[0], "w1": w[1], "w2": w[2],
        "be0": be[0], "be1": be[1], "be2": be[2],
    }
    in_maps = []
    for i in range(NCORES):
        m = dict(shared)
        m["xgT"] = np.ascontiguousarray(xgT[:, i * BL : (i + 1) * BL])
        m["xm"] = np.ascontiguousarray(xmp[i * BL : (i + 1) * BL])
        in_maps.append(m)
    return in_maps


def kernel(**inputs):
    from concourse.bass_utils import run_bass_kernel_spmd

    if "nc" not in _cache:
        _cache["nc"] = _build_nc()
    nc = _cache["nc"]

    in_maps = _prep_inputs(inputs)
    res = run_bass_kernel_spmd(nc, in_maps, core_ids=list(range(NCORES)))
    out = np.concatenate([r["out"] for r in res.results], axis=0)
    return np.ascontiguousarray(out.astype(np.float32))


# revision 10
# speedup vs baseline: 1.4517x; 1.4517x over previous
# Trainium2 Bass kernel for dense soft-MoE (nn_MANN_78726750536045).
#
# Math (per sample b):
#   gates = softmax(MLP_elu(x_gate))                     [K=8]
#   h0 = elu(sum_k g_k * (x_main @ W1_k.T) + gates@eb1)  [512]
#   h1 = elu(sum_k g_k * (h0 @ W2_k.T) + gates@eb2)      [512]
#   out =     sum_k g_k * (h1 @ W3_k.T) + gates@eb3      [512]
#
# Key transformation: g_k * (h @ Wk.T) == ((g_k * h) @ Wk.T), so each expert
# layer becomes ONE dense GEMM with contraction dim K*512 = 4096 over the
# "gated-replicated" activation X'[(k,i), b] = g[b,k] * h[b,i].  X' is built
# on the tensor engine itself via per-expert diagonal matrices:
#   X'_chunk = h_chunk.T @ diag(g_k)   (fuses the transpose AND the gating).
# The bias gates@eb is folded in as one extra 8-row chunk of the same GEMM.
#
# Sharding: pure data-parallel, batch 1024 -> 128 rows per core x 8 cores.
# Expert weights are cast to bf16 on host (HBM traffic is the bottleneck:
# ~12.3MB/core bf16 @ ~360GB/s ~= 34us); gating network stays fp32.

import numpy as np
import ml_dtypes

B = 1024
X_MAIN, X_GATE, HID, Y_DIM, GHID, K = 480, 128, 512, 512, 32, 8
NCORES = 8
BL = B // NCORES  # 128 rows per core
P = 128
NCH = 32  # contraction chunks per expert layer (K * 512 / 128)

_cache = {}


def _build_nc():
    from contextlib import ExitStack

    import concourse.bacc as bacc
    import concourse.mybir as mybir
    import concourse.tile as tile
    from concourse.bass import ts

    f32 = mybir.dt.float32
    bf16 = mybir.dt.bfloat16
    AF = mybir.ActivationFunctionType
    OP = mybir.AluOpType

    nc = bacc.Bacc("TRN2", target_bir_lowering=False, debug=False)

    # ---- DRAM I/O ----
    d_xgT = nc.dram_tensor("xgT", [X_GATE, BL], f32, kind="ExternalInput")
    d_xm = nc.dram_tensor("xm", [BL, HID], bf16, kind="ExternalInput")
    d_g1T = nc.dram_tensor("g1T", [X_GATE, GHID], f32, kind="ExternalInput")
    d_g2T = nc.dram_tensor("g2T", [GHID, GHID], f32, kind="ExternalInput")
    d_g3T = nc.dram_tensor("g3T", [GHID, K], f32, kind="ExternalInput")
    d_gb1 = nc.dram_tensor("gb1", [GHID, 1], f32, kind="ExternalInput")
    d_gb2 = nc.dram_tensor("gb2", [GHID, 1], f32, kind="ExternalInput")
    d_gb3 = nc.dram_tensor("gb3", [K, 1], f32, kind="ExternalInput")
    d_idf = nc.dram_tensor("idf", [P, P], f32, kind="ExternalInput")
    # weights packed per-partition-contiguous: w[p, c*512 + o] = W'[c*128+p, o]
    d_w = [
        nc.dram_tensor(f"w{l}", [P, NCH * HID], bf16, kind="ExternalInput")
        for l in range(3)
    ]
    d_be = [
        nc.dram_tensor(f"be{l}", [K, HID], bf16, kind="ExternalInput")
        for l in range(3)
    ]
    d_out = nc.dram_tensor("out", [BL, Y_DIM], f32, kind="ExternalOutput")

    with ExitStack() as ctx:
        tc = ctx.enter_context(tile.TileContext(nc))
        consts = ctx.enter_context(tc.tile_pool(name="consts", bufs=1))
        sb = ctx.enter_context(tc.tile_pool(name="sb", bufs=2))
        xpp = ctx.enter_context(tc.tile_pool(name="xpp", bufs=2))
        pmain = ctx.enter_context(tc.tile_pool(name="pmain", bufs=2, space="PSUM"))
        ppx = ctx.enter_context(tc.tile_pool(name="ppx", bufs=4, space="PSUM"))
        pg = ctx.enter_context(tc.tile_pool(name="pg", bufs=2, space="PSUM"))

        # ---- load constants ----
        t_xgT = consts.tile([X_GATE, BL], f32)
        nc.sync.dma_start(t_xgT, d_xgT[:])
        t_xm = consts.tile([BL, HID], bf16)
        nc.sync.dma_start(t_xm, d_xm[:])
        t_g1T = consts.tile([X_GATE, GHID], f32)
        nc.sync.dma_start(t_g1T, d_g1T[:])
        t_g2T = consts.tile([GHID, GHID], f32)
        nc.sync.dma_start(t_g2T, d_g2T[:])
        t_g3T = consts.tile([GHID, K], f32)
        nc.sync.dma_start(t_g3T, d_g3T[:])
        t_gb1 = consts.tile([GHID, 1], f32)
        nc.sync.dma_start(t_gb1, d_gb1[:])
        t_gb2 = consts.tile([GHID, 1], f32)
        nc.sync.dma_start(t_gb2, d_gb2[:])
        t_gb3 = consts.tile([K, 1], f32)
        nc.sync.dma_start(t_gb3, d_gb3[:])
        t_idf = consts.tile([P, P], f32)
        nc.sync.dma_start(t_idf, d_idf[:])
        t_be = []
        for l in range(3):
            tb = consts.tile([K, HID], bf16, name=f"be{l}")
            nc.sync.dma_start(tb, d_be[l][:])
            t_be.append(tb)

        # Preload all expert weights into SBUF: one big tile per layer,
        # 4 DMA pieces each (128 x 8KB contiguous descriptors), alternating
        # between the SP and Activation HWDGE rings for parallelism.
        NPC = 8  # chunks per DMA piece
        t_w = []
        for l in range(3):
            wt = consts.tile([P, NCH * HID], bf16, name=f"wl{l}")
            for q in range(NCH // NPC):
                sl = slice(q * NPC * HID, (q + 1) * NPC * HID)
                eng = nc.sync if (l * 4 + q) % 2 == 0 else nc.scalar
                eng.dma_start(wt[:, sl], d_w[l][:, sl])
            t_w.append(wt)

        # ---- gating network (fp32, [feature, batch] layout) ----
        def elu_block(p_in, bias, width):
            e = sb.tile([width, BL], f32, tag="gelu_e")
            nc.scalar.activation(e, p_in, AF.Exp, bias=bias)
            r = sb.tile([width, BL], f32, tag="gelu_r")
            nc.vector.tensor_scalar(r, p_in, bias, 0.0, OP.add, OP.max)
            t = sb.tile([width, BL], f32, tag="gelu_t")
            nc.vector.tensor_scalar(t, e, -1.0, 0.0, OP.add, OP.min)
            g = sb.tile([width, BL], f32, tag="gelu_g")
            nc.vector.tensor_tensor(g, r, t, OP.add)
            return g

        p1 = pg.tile([GHID, BL], f32, tag="pg")
        nc.tensor.matmul(p1, t_g1T, t_xgT, start=True, stop=True)
        g1 = elu_block(p1, t_gb1, GHID)

        p2 = pg.tile([GHID, BL], f32, tag="pg")
        nc.tensor.matmul(p2, t_g2T, g1, start=True, stop=True)
        g2 = elu_block(p2, t_gb2, GHID)

        p3 = pg.tile([K, BL], f32, tag="pg")
        nc.tensor.matmul(p3, t_g3T, g2, start=True, stop=True)

        # softmax over K (partition dim): exp -> transpose [K,BL]->[BL,K]
        # -> free-dim sum + reciprocal + scale.
        es = sb.tile([K, BL], f32)
        nc.scalar.activation(es, p3, AF.Exp, bias=t_gb3)
        p_esT = pg.tile([BL, K], f32, tag="pg")
        nc.tensor.transpose(p_esT, es, t_idf[0:K, 0:K])
        ssum = sb.tile([BL, 1], f32)
        nc.vector.tensor_reduce(ssum, p_esT, mybir.AxisListType.X, OP.add)
        recip = sb.tile([BL, 1], f32)
        nc.vector.reciprocal(recip, ssum)
        gates = sb.tile([BL, K], f32)  # [b, k] fp32
        nc.vector.tensor_scalar(gates, p_esT, recip, None, OP.mult)

        # gates^T [K, BL] in bf16 (for the bias chunk of the main GEMM)
        p_gT = pg.tile([K, BL], f32, tag="pg")
        nc.tensor.transpose(p_gT, gates, t_idf)
        gT = sb.tile([K, BL], bf16)
        nc.vector.tensor_copy(gT, p_gT)

        # per-expert diagonal matrices diag(g[:,k]), bf16, built once
        diags = consts.tile([P, K * P], bf16)
        for k in range(K):
            nc.vector.tensor_scalar(
                diags[:, ts(k, P)], t_idf, gates[:, k : k + 1], None, OP.mult
            )

        # ---- three expert layers ----
        h = t_xm
        for l in range(3):
            # Build X' [(j,k,il), b] in SBUF: for each input-block j (128 wide)
            # and each group g of 4 experts, one matmul
            #   px = h_j.T @ [diag_{4g} .. diag_{4g+3}]  -> [128, 512]
            XP = xpp.tile([P, NCH * P], bf16, tag="XP")
            for j in range(4):
                for g in range(2):
                    px = ppx.tile([P, 512], f32, tag="px")
                    nc.tensor.matmul(
                        px,
                        h[:, ts(j, P)],
                        diags[:, g * 512 : (g + 1) * 512],
                        start=True,
                        stop=True,
                    )
                    off = (j * 8 + g * 4) * P
                    if (j * 2 + g) % 2 == 0:
                        nc.vector.tensor_copy(XP[:, off : off + 512], px)
                    else:
                        nc.scalar.copy(XP[:, off : off + 512], px)

            # Main GEMM: 32 accumulating chunk matmuls + 1 bias chunk
            pm = pmain.tile([P, HID], f32, tag="pm")
            for c in range(NCH):
                nc.tensor.matmul(
                    pm,
                    XP[:, ts(c, P)],
                    t_w[l][:, c * HID : (c + 1) * HID],
                    start=(c == 0),
                    stop=False,
                )
            nc.tensor.matmul(pm, gT, t_be[l], start=False, stop=True)

            if l < 2:
                # ELU: relu(x) + min(exp(x)-1, 0), output bf16
                e = sb.tile([P, HID], f32, tag="ee")
                nc.scalar.activation(e, pm, AF.Exp)
                r = sb.tile([P, HID], f32, tag="er")
                nc.scalar.activation(r, pm, AF.Relu)
                t = sb.tile([P, HID], f32, tag="et")
                nc.vector.tensor_scalar(t, e, -1.0, 0.0, OP.add, OP.min)
                h2 = sb.tile([P, HID], bf16, tag="eh")
                nc.vector.tensor_tensor(h2, r, t, OP.add)
                h = h2
            else:
                o = sb.tile([P, Y_DIM], f32)
                nc.vector.tensor_copy(o, pm)
                nc.sync.dma_start(d_out[:], o)

    nc.compile()
    return nc


def _prep_inputs(inputs):
    bf16 = ml_dtypes.bfloat16
    xm = np.asarray(inputs["x_main"], np.float32)
    xg = np.asarray(inputs["x_gate"], np.float32)

    xgT = np.ascontiguousarray(xg.T)  # [128, B]
    xmp = np.zeros((B, HID), np.float32)
    xmp[:, :X_MAIN] = xm
    xmp = xmp.astype(bf16)

    g1T = np.ascontiguousarray(np.asarray(inputs["gw1"], np.float32).T)
    g2T = np.ascontiguousarray(np.asarray(inputs["gw2"], np.float32).T)
    g3T = np.ascontiguousarray(np.asarray(inputs["gw3"], np.float32).T)
    gb1 = np.asarray(inputs["gb1"], np.float32).reshape(GHID, 1)
    gb2 = np.asarray(inputs["gb2"], np.float32).reshape(GHID, 1)
    gb3 = np.asarray(inputs["gb3"], np.float32).reshape(K, 1)
    idf = np.eye(P, dtype=np.float32)

    # expert weights -> per-partition-contiguous chunk layout, bf16:
    # w[p, (j*8+k)*512 + o] = ew[k][o, j*128+p]  (input dim zero-padded to 512)
    def pack_w(ew):
        ewt = np.asarray(ew, np.float32).transpose(0, 2, 1)  # [K, in, out]
        if ewt.shape[1] < HID:
            pad = np.zeros((K, HID, ewt.shape[2]), np.float32)
            pad[:, : ewt.shape[1], :] = ewt
            ewt = pad
        # dims (k, j, p, o) -> (p, j, k, o) -> [128, 4*8*512]
        w = ewt.reshape(K, 4, P, HID).transpose(2, 1, 0, 3).reshape(P, NCH * HID)
        return np.ascontiguousarray(w.astype(bf16))

    w = [pack_w(inputs["ew1"]), pack_w(inputs["ew2"]), pack_w(inputs["ew3"])]
    be = [
        np.asarray(inputs[f"eb{l + 1}"], np.float32).astype(bf16) for l in range(3)
    ]

    shared = {
        "g1T": g1T, "g2T": g2T, "g3T": g3T,
        "gb1": gb1, "gb2": gb2, "gb3": gb3, "idf": idf,
        "w0": w[0], "w1": w[1], "w2": w[2],
        "be0": be[0], "be1": be[1], "be2": be[2],
    }
    in_maps = []
    for i in range(NCORES):
        m = dict(shared)
        m["xgT"] = np.ascontiguousarray(xgT[:, i * BL : (i + 1) * BL])
        m["xm"] = np.ascontiguousarray(xmp[i * BL : (i + 1) * BL])
        in_maps.append(m)
    return in_maps


def kernel(**inputs):
    from concourse.bass_utils import run_bass_kernel_spmd

    if "nc" not in _cache:
        _cache["nc"] = _build_nc()
    nc = _cache["nc"]

    in_maps = _prep_inputs(inputs)
    res = run_bass_kernel_spmd(nc, in_maps, core_ids=list(range(NCORES)))
    out = np.concatenate([r["out"] for r in res.results], axis=0)
    return np.ascontiguousarray(out.astype(np.float32))
